# revision 12
# baseline (speedup 1.0000x reference)
"""MoE layer (8 experts, top-2) on 8 Trainium2 NeuronCores.

Strategy (expert parallelism, per the sharding hint):
  Launch 1 (router): tokens data-parallel across the 8 cores, each core
    computes its slice of router logits in true fp32 on the PE.
  Host dispatch:     softmax/top-2/combine-weights replicated from the
    reference in fp32 on the host (0.3 MFLOP of control logic), tokens
    gathered per expert (capacity padded).
  Launch 2 (experts): core e holds expert e's weights; computes
    y = (relu(x @ W1^T)^2 @ W2^T) * w for its gathered tokens.
    Matmuls run in fp16 (fp32 PSUM accumulation); weight loads overlap
    with streaming because 2-byte stationaries use the background
    weight buffer.
  Host combine:      scatter-add of the two expert contributions per
    token, ascending expert order (same fp32 summation order as the
    reference loop).

All matmul FLOPs run on device. Host does data movement + top-2 dispatch.
"""

import numpy as np

N_EXPERTS = 8
TOP_K = 2
N_EMBD = 1024
EXPERT_DIM = 2048
N_TOKENS = 8192          # 4 * 2048
N_CORES = 8
TOK_PER_CORE = N_TOKENS // N_CORES  # 1024 (router shard)
CAP = 2176               # per-expert token capacity (17*128; max observed
                         # count is 2175 for the fixed seed). If routing ever
                         # assigns more than CAP tokens to one expert, the
                         # host runs a second expert pass for the overflow
                         # (correct for any input, never triggered here).
TCH = 256                # expert-kernel token chunk (multiple of 128, max 512
                         # = fp32 PSUM bank limit on the matmul free dim;
                         # 256 measured fastest)

_CACHE = {}


def _build_router_module(repeat=1, dtype="f16"):
    """Computes logitsT [E, T] = router_w @ x^T (fp32 PSUM accumulation).

    x streams in fp16 (half the DMA of fp32; routing flips are repaired
    by the host margin fixup in kernel()). Per-k DMA granularity so the
    PE overlaps with the x load; lhsT = rw [d, E] stationaries are tiny
    (8-row loads).
    """
    import concourse.bacc as bacc
    import concourse.mybir as mybir
    import concourse.tile as tile

    f32 = mybir.dt.float32
    dt_x = mybir.dt.float16 if dtype == "f16" else f32
    D = N_EMBD
    E = N_EXPERTS
    T = TOK_PER_CORE
    KC = D // 128   # 8 contraction chunks
    TT = 512        # moving-tile token width
    NT = T // TT    # 2 token tiles

    nc = bacc.Bacc("TRN2", target_bir_lowering=False, debug=False,
                   num_devices=N_CORES)
    xT = nc.dram_tensor("xT", [D, T], dt_x, kind="ExternalInput").ap()
    rwT = nc.dram_tensor("rwT", [D, E], dt_x, kind="ExternalInput").ap()
    logitsT = nc.dram_tensor("logitsT", [E, T], f32, kind="ExternalOutput").ap()

    with tile.TileContext(nc) as tc:
        with (
            tc.tile_pool(name="wpool", bufs=1) as wpool,
            tc.tile_pool(name="xpool", bufs=2) as xpool,
            tc.tile_pool(name="opool", bufs=2) as opool,
            tc.tile_pool(name="pspool", bufs=2, space="PSUM") as pspool,
        ):
            # router weights: block k = rwT[k*128:(k+1)*128, :]
            rw_tile = wpool.tile([128, KC, E], dt_x, tag="rw")
            for k in range(KC):
                nc.sync.dma_start(rw_tile[:, k, :],
                                  rwT[k * 128:(k + 1) * 128, :])

            def body(_=None):
                # per-k x tiles so matmuls start as soon as each k lands
                x_tiles = []
                for k in range(KC):
                    xk = xpool.tile([128, T], dt_x, tag=f"x{k}", name=f"x{k}")
                    nc.sync.dma_start(xk[:], xT[k * 128:(k + 1) * 128, :])
                    x_tiles.append(xk)
                pls = [pspool.tile([E, TT], f32, tag=f"pl{tt}", name=f"pl_{tt}")
                       for tt in range(NT)]
                for k in range(KC):
                    for tt in range(NT):
                        nc.tensor.matmul(
                            pls[tt][:],
                            rw_tile[:, k, :],                            # lhsT [d, e]
                            x_tiles[k][:, tt * TT:(tt + 1) * TT],        # rhs [d, t]
                            start=(k == 0), stop=(k == KC - 1))
                for tt in range(NT):
                    ot = opool.tile([E, TT], f32, tag=f"o{tt}", name=f"o_{tt}")
                    nc.scalar.copy(ot[:], pls[tt][:])
                    nc.sync.dma_start(logitsT[:, tt * TT:(tt + 1) * TT], ot[:])

            if repeat == 1:
                body()
            else:
                with tc.For_i(0, repeat, 1) as _i:
                    body(_i)
    nc.compile()
    return nc


def _build_expert_module(repeat=1, mm1="f32r", mm2="f32r", tch=None,
                         interleave=False, act_mode="relu2", skip_ydma=False,
                         load_x_once=False, ph_bufs=4, py_bufs=3,
                         x_layout="plain", x_bufs=3):
    import concourse.bacc as bacc
    import concourse.mybir as mybir
    import concourse.tile as tile

    if tch is None:
        tch = TCH
    f32 = mybir.dt.float32
    dt_mm1 = mybir.dt.float32r if mm1 == "f32r" else mybir.dt.float16
    dt_mm2 = mybir.dt.float32r if mm2 == "f32r" else mybir.dt.float16
    D = N_EMBD
    F = EXPERT_DIM
    KD = D // 128     # 8 d-chunks
    KF = F // 128     # 16 f-chunks

    nc = bacc.Bacc("TRN2", target_bir_lowering=False, debug=False,
                   num_devices=N_CORES)
    if x_layout == "blocked":
        xT = nc.dram_tensor("xT", [128, KD * CAP], dt_mm1,
                            kind="ExternalInput").ap()
    else:
        xT = nc.dram_tensor("xT", [D, CAP], dt_mm1, kind="ExternalInput").ap()
    w1T = nc.dram_tensor("w1T", [D, F], dt_mm1, kind="ExternalInput").ap()
    w2T = nc.dram_tensor("w2T", [F, D], dt_mm2, kind="ExternalInput").ap()
    wv = nc.dram_tensor("wv", [CAP, 1], f32, kind="ExternalInput").ap()
    y = nc.dram_tensor("y", [CAP, D], f32, kind="ExternalOutput").ap()

    with tile.TileContext(nc) as tc:
        with (
            tc.tile_pool(name="wpool", bufs=1) as wpool,
            tc.tile_pool(name="xpool", bufs=x_bufs) as xpool,
            tc.tile_pool(name="hpool", bufs=2) as hpool,
            tc.tile_pool(name="rpool", bufs=3) as rpool,
            tc.tile_pool(name="ypool", bufs=3) as ypool,
            tc.tile_pool(name="ph_pool", bufs=ph_bufs, space="PSUM") as ph_pool,
            tc.tile_pool(name="py_pool", bufs=py_bufs, space="PSUM") as py_pool,
        ):
            # chunk list: (token base, chunk width); widths are multiples
            # of 128 and at most 512 (fp32 PSUM bank limit on N)
            chunks = []
            base = 0
            while base < CAP:
                w = min(tch, CAP - base)
                chunks.append((base, w))
                base += w

            def load_x_chunk(c, cb, cw):
                # x^T chunk: block k = [128(d), cw tokens]
                x_tile = xpool.tile([128, KD * cw], dt_mm1, tag="x",
                                    name=f"x_{c}")
                if x_layout == "blocked":
                    # host pre-blocked: chunk c contiguous, 1 DMA, 4KB rows
                    nc.sync.dma_start(x_tile[:, :],
                                      xT[0:128, KD * cb:KD * (cb + cw)])
                else:
                    for k in range(KD):
                        nc.sync.dma_start(
                            x_tile[:, k * cw:(k + 1) * cw],
                            xT[k * 128:(k + 1) * 128, cb:cb + cw])
                return x_tile

            # --- resident weights ---
            # DMA issue order shapes the queue order: first the W1 column
            # slices chunk 0's mm1 needs, then chunk 0's x, then the rest
            # (W2 is first consumed ~25us in, after chunk 0's mm1).
            # W1^T d-chunk k: [128(d), F]
            w1_tiles = [
                wpool.tile([128, F], dt_mm1, tag=f"w1_{k}", name=f"w1_{k}")
                for k in range(KD)
            ]
            x0_tile = load_x_chunk(0, chunks[0][0], chunks[0][1])
            for q in range(4):
                for k in range(KD):
                    nc.sync.dma_start(
                        w1_tiles[k][:, q * (F // 4):(q + 1) * (F // 4)],
                        w1T[k * 128:(k + 1) * 128, q * (F // 4):(q + 1) * (F // 4)])
                if q == 0:
                    x1_tile = load_x_chunk(1, chunks[1][0], chunks[1][1])
            # W2^T f-chunk k: [128(f), D]
            w2_tiles = []
            for k in range(KF):
                t = wpool.tile([128, D], dt_mm2, tag=f"w2_{k}", name=f"w2_{k}")
                nc.sync.dma_start(t[:], w2T[k * 128:(k + 1) * 128, :])
                w2_tiles.append(t)
            # combine weights: column j = tokens [j*128, (j+1)*128)
            wv_tile = wpool.tile([128, CAP // 128], f32, tag="wv", name="wv")
            for j in range(CAP // 128):
                nc.sync.dma_start(wv_tile[:, j:j + 1],
                                  wv[j * 128:(j + 1) * 128, :])

            def fc1_part(c, cb, cw, x_tile):
                # h^T chunk: block f = [128(f), cw]
                h_tile = hpool.tile([128, KF * cw], dt_mm2, tag="h",
                                    name=f"h_{c}")
                for f in range(KF):
                    ph = ph_pool.tile([128, cw], f32, tag="ph",
                                      name=f"ph_{c}_{f}")
                    for k in range(KD):
                        nc.tensor.matmul(
                            ph[:],
                            w1_tiles[k][:, f * 128:(f + 1) * 128],
                            x_tile[:, k * cw:(k + 1) * cw],
                            start=(k == 0), stop=(k == KD - 1))
                    if act_mode == "relu2":
                        hr = rpool.tile([128, cw], f32, tag="hr",
                                        name=f"hr_{c}_{f}")
                        nc.vector.tensor_scalar_max(hr[:], ph[:], 0.0)
                        nc.scalar.square(h_tile[:, f * cw:(f + 1) * cw], hr[:])
                    else:  # copy-only: same deps, no vector op
                        nc.scalar.copy(h_tile[:, f * cw:(f + 1) * cw], ph[:])
                return h_tile

            def fc2_part(c, cb, cw, h_tile):
                for s in range(cw // 128):
                    yt = ypool.tile([128, D], f32, tag="y",
                                    name=f"y_{c}_{s}")
                    for dn in range(D // 512):
                        py = py_pool.tile([128, 512], f32, tag="py",
                                          name=f"py_{c}_{s}_{dn}")
                        for f in range(KF):
                            nc.tensor.matmul(
                                py[:],
                                h_tile[:, f * cw + s * 128:
                                       f * cw + (s + 1) * 128],
                                w2_tiles[f][:, dn * 512:(dn + 1) * 512],
                                start=(f == 0), stop=(f == KF - 1))
                        nc.scalar.mul(yt[:, dn * 512:(dn + 1) * 512], py[:],
                                      wv_tile[:, (cb + s * 128) // 128:
                                              (cb + s * 128) // 128 + 1])
                    if not skip_ydma:
                        nc.sync.dma_start(
                            y[cb + s * 128:cb + (s + 1) * 128, :],
                            yt[:])

            def body(_=None, preloaded=()):
                if interleave:
                    pending = None  # (c, cb, cw, h_tile) awaiting fc2
                    for c, (cb, cw) in enumerate(chunks):
                        if c < len(preloaded):
                            x_tile = preloaded[c]
                        else:
                            x_tile = load_x_chunk(c, cb, cw)
                        h_tile = fc1_part(c, cb, cw, x_tile)
                        if pending is not None:
                            fc2_part(*pending)
                        pending = (c, cb, cw, h_tile)
                    fc2_part(*pending)
                else:
                    for c, (cb, cw) in enumerate(chunks):
                        if c < len(preloaded):
                            x_tile = preloaded[c]
                        else:
                            x_tile = load_x_chunk(c, cb, cw)
                        h_tile = fc1_part(c, cb, cw, x_tile)
                        fc2_part(c, cb, cw, h_tile)

            if repeat == 1:
                body(preloaded=(x0_tile, x1_tile))
            else:
                if load_x_once:
                    pre = [x0_tile, x1_tile] + [
                        load_x_chunk(c, cb, cw)
                        for c, (cb, cw) in enumerate(chunks) if c >= 2]
                    with tc.For_i(0, repeat, 1) as _i:
                        body(_i, preloaded=tuple(pre))
                else:
                    with tc.For_i(0, repeat, 1) as _i:
                        body(_i)
    nc.compile()
    return nc


CAP_A = 2176   # F-split slot A capacity (largest-4 experts)
CAP_B = 2048   # F-split slot B capacity (smallest-4 experts)
FH = EXPERT_DIM // 2


def _build_expert2_module(repeat=1, mm1="f16", mm2="f16", tch=None):
    """F-split expert pair: each core runs one F-half of a 'big' expert
    (CAP_A token slots) and one F-half of a 'small' expert (CAP_B slots).

    Per-core equivalent work drops from CAP tokens to (CAP_A+CAP_B)/2 by
    pairing hot with cold experts; partial y outputs (over half the
    hidden dim) are summed on the host during combine.
    """
    import concourse.bacc as bacc
    import concourse.mybir as mybir
    import concourse.tile as tile

    if tch is None:
        tch = TCH
    f32 = mybir.dt.float32
    dt_mm1 = mybir.dt.float32r if mm1 == "f32r" else mybir.dt.float16
    dt_mm2 = mybir.dt.float32r if mm2 == "f32r" else mybir.dt.float16
    D = N_EMBD
    KD = D // 128      # 8 d-chunks
    KFH = FH // 128    # 8 f-chunks per half

    nc = bacc.Bacc("TRN2", target_bir_lowering=False, debug=False,
                   num_devices=N_CORES)
    slots_io = {}
    for s, cap in (("a", CAP_A), ("b", CAP_B)):
        slots_io[s] = {
            "xT": nc.dram_tensor(f"xT{s}", [D, cap], dt_mm1,
                                 kind="ExternalInput").ap(),
            "w1T": nc.dram_tensor(f"w1T{s}", [D, FH], dt_mm1,
                                  kind="ExternalInput").ap(),
            "w2T": nc.dram_tensor(f"w2T{s}", [FH, D], dt_mm2,
                                  kind="ExternalInput").ap(),
            "wv": nc.dram_tensor(f"wv{s}", [cap, 1], f32,
                                 kind="ExternalInput").ap(),
            "y": nc.dram_tensor(f"y{s}", [cap, D], f32,
                                kind="ExternalOutput").ap(),
            "cap": cap,
        }

    with tile.TileContext(nc) as tc:
        with (
            tc.tile_pool(name="wpool", bufs=1) as wpool,
            tc.tile_pool(name="xpool", bufs=3) as xpool,
            tc.tile_pool(name="hpool", bufs=2) as hpool,
            tc.tile_pool(name="rpool", bufs=3) as rpool,
            tc.tile_pool(name="ypool", bufs=3) as ypool,
            tc.tile_pool(name="ph_pool", bufs=4, space="PSUM") as ph_pool,
            tc.tile_pool(name="py_pool", bufs=3, space="PSUM") as py_pool,
        ):
            slots = {}
            for s in ("a", "b"):
                io = slots_io[s]
                cap = io["cap"]
                chunks = []
                base = 0
                while base < cap:
                    w = min(tch, cap - base)
                    chunks.append((base, w))
                    base += w
                slots[s] = {"io": io, "chunks": chunks}

            def load_x_chunk(s, c, cb, cw):
                io = slots[s]["io"]
                x_tile = xpool.tile([128, KD * cw], dt_mm1, tag=f"x{s}",
                                    name=f"x{s}_{c}")
                for k in range(KD):
                    nc.sync.dma_start(
                        x_tile[:, k * cw:(k + 1) * cw],
                        io["xT"][k * 128:(k + 1) * 128, cb:cb + cw])
                return x_tile

            # --- resident weights; slot A's first f-slices land first ---
            for s in ("a", "b"):
                io = slots[s]["io"]
                slots[s]["w1_tiles"] = [
                    wpool.tile([128, FH], dt_mm1, tag=f"w1{s}_{k}",
                               name=f"w1{s}_{k}")
                    for k in range(KD)
                ]
                slots[s]["w2_tiles"] = [
                    wpool.tile([128, D], dt_mm2, tag=f"w2{s}_{k}",
                               name=f"w2{s}_{k}")
                    for k in range(KFH)
                ]
                cap = io["cap"]
                t = wpool.tile([128, cap // 128], f32, tag=f"wv{s}",
                               name=f"wv{s}")
                slots[s]["wv_tile"] = t

            for q in range(2):
                for k in range(KD):
                    nc.sync.dma_start(
                        slots["a"]["w1_tiles"][k][:, q * (FH // 2):(q + 1) * (FH // 2)],
                        slots_io["a"]["w1T"][k * 128:(k + 1) * 128,
                                             q * (FH // 2):(q + 1) * (FH // 2)])
                if q == 0:
                    xa0 = load_x_chunk("a", 0, *slots["a"]["chunks"][0])
                    xa1 = load_x_chunk("a", 1, *slots["a"]["chunks"][1])
            for k in range(KFH):
                nc.sync.dma_start(slots["a"]["w2_tiles"][k][:],
                                  slots_io["a"]["w2T"][k * 128:(k + 1) * 128, :])
            for k in range(KD):
                nc.sync.dma_start(slots["b"]["w1_tiles"][k][:],
                                  slots_io["b"]["w1T"][k * 128:(k + 1) * 128, :])
            for k in range(KFH):
                nc.sync.dma_start(slots["b"]["w2_tiles"][k][:],
                                  slots_io["b"]["w2T"][k * 128:(k + 1) * 128, :])
            for s in ("a", "b"):
                io = slots[s]["io"]
                for j in range(io["cap"] // 128):
                    nc.sync.dma_start(slots[s]["wv_tile"][:, j:j + 1],
                                      io["wv"][j * 128:(j + 1) * 128, :])

            def fc1_part(s, c, cb, cw, x_tile):
                sl = slots[s]
                h_tile = hpool.tile([128, KFH * cw], dt_mm2, tag="h",
                                    name=f"h{s}_{c}")
                for f in range(KFH):
                    ph = ph_pool.tile([128, cw], f32, tag="ph",
                                      name=f"ph{s}_{c}_{f}")
                    for k in range(KD):
                        nc.tensor.matmul(
                            ph[:],
                            sl["w1_tiles"][k][:, f * 128:(f + 1) * 128],
                            x_tile[:, k * cw:(k + 1) * cw],
                            start=(k == 0), stop=(k == KD - 1))
                    hr = rpool.tile([128, cw], f32, tag="hr",
                                    name=f"hr{s}_{c}_{f}")
                    nc.vector.tensor_scalar_max(hr[:], ph[:], 0.0)
                    nc.scalar.square(h_tile[:, f * cw:(f + 1) * cw], hr[:])
                return h_tile

            def fc2_part(s, c, cb, cw, h_tile):
                sl = slots[s]
                io = sl["io"]
                for t in range(cw // 128):
                    yt = ypool.tile([128, D], f32, tag="y",
                                    name=f"y{s}_{c}_{t}")
                    for dn in range(D // 512):
                        py = py_pool.tile([128, 512], f32, tag="py",
                                          name=f"py{s}_{c}_{t}_{dn}")
                        for f in range(KFH):
                            nc.tensor.matmul(
                                py[:],
                                h_tile[:, f * cw + t * 128:
                                       f * cw + (t + 1) * 128],
                                sl["w2_tiles"][f][:, dn * 512:(dn + 1) * 512],
                                start=(f == 0), stop=(f == KFH - 1))
                        nc.scalar.mul(yt[:, dn * 512:(dn + 1) * 512], py[:],
                                      sl["wv_tile"][:, (cb + t * 128) // 128:
                                                    (cb + t * 128) // 128 + 1])
                    nc.sync.dma_start(
                        io["y"][cb + t * 128:cb + (t + 1) * 128, :],
                        yt[:])

            def body(_=None, preloaded=()):
                items = ([("a", c, cb, cw) for c, (cb, cw)
                          in enumerate(slots["a"]["chunks"])]
                         + [("b", c, cb, cw) for c, (cb, cw)
                            in enumerate(slots["b"]["chunks"])])
                pending = None
                for i, (s, c, cb, cw) in enumerate(items):
                    if i < len(preloaded):
                        x_tile = preloaded[i]
                    else:
                        x_tile = load_x_chunk(s, c, cb, cw)
                    h_tile = fc1_part(s, c, cb, cw, x_tile)
                    if pending is not None:
                        fc2_part(*pending)
                    pending = (s, c, cb, cw, h_tile)
                fc2_part(*pending)

            if repeat == 1:
                body(preloaded=(xa0, xa1))
            else:
                with tc.For_i(0, repeat, 1) as _i:
                    body(_i)
    nc.compile()
    return nc


# dtype plan for the two expert matmuls: "f32r" (FP22) or "f16".
# fp16 RNE quantization measures only ~2x the error of f32r truncation
# (rel 4.3e-4 vs 2.1e-4) and allows standalone, overlapped weight loads
# (4-byte matmuls pay a serial ~107ns self-load per matmul).
EXPERT_MM1 = "f16"
EXPERT_MM2 = "f16"
EXPERT_SPLIT = True   # F-split hot/cold expert pairing (falls back if
                      # the per-slot capacities don't fit the routing)


def _get_module(name):
    if name not in _CACHE:
        if name == "router":
            _CACHE[name] = _build_router_module()
        elif name == "expert":
            _CACHE[name] = _build_expert_module(mm1=EXPERT_MM1, mm2=EXPERT_MM2,
                                                interleave=True)
        elif name == "expert2":
            _CACHE[name] = _build_expert2_module(mm1=EXPERT_MM1, mm2=EXPERT_MM2)
        else:
            raise KeyError(name)
    return _CACHE[name]


def _routing_from_logits(logits):
    """Replicates reference softmax/top-2/normalize in fp32 numpy.

    jax.lax.top_k tie-break (lower index first) == stable argsort on -p.
    """
    logits = logits.astype(np.float32, copy=False)
    m = logits.max(axis=1, keepdims=True)
    p = np.exp(logits - m)
    p = (p / p.sum(axis=1, keepdims=True)).astype(np.float32)
    order = np.argsort(-p, axis=1, kind="stable")
    t1 = order[:, 0].astype(np.int32)
    t2 = order[:, 1].astype(np.int32)
    ar = np.arange(logits.shape[0])
    tv1 = p[ar, t1]
    tv2 = p[ar, t2]
    s = (tv1 + tv2).astype(np.float32)
    w1 = (tv1 / s).astype(np.float32)
    w2 = (tv2 / s).astype(np.float32)
    return t1, t2, w1, w2


def kernel(x, router_w, fc1_w, fc2_w):
    from concourse.bass_utils import run_bass_kernel_spmd

    x = np.ascontiguousarray(np.asarray(x, dtype=np.float32))
    router_w = np.ascontiguousarray(np.asarray(router_w, dtype=np.float32))
    fc1_w = np.asarray(fc1_w, dtype=np.float32)
    fc2_w = np.asarray(fc2_w, dtype=np.float32)

    B, T, D = x.shape
    xf = x.reshape(B * T, D)
    xT = np.ascontiguousarray(xf.T)               # [D, N]
    xT16 = np.ascontiguousarray(xT.astype(np.float16))
    rwT16 = np.ascontiguousarray(router_w.T.astype(np.float16))

    # --- launch 1: router logits on device (fp16 in, fp32 accumulate) ---
    nc_r = _get_module("router")
    in_maps = [
        {"xT": np.ascontiguousarray(xT16[:, c * TOK_PER_CORE:(c + 1) * TOK_PER_CORE]),
         "rwT": rwT16}
        for c in range(N_CORES)
    ]
    res = run_bass_kernel_spmd(nc_r, in_maps, core_ids=list(range(N_CORES)))
    logits = np.concatenate(
        [np.ascontiguousarray(r["logitsT"].T) for r in res.results], axis=0)

    # Host margin fixup: tokens whose top-3 logit gaps are within 10x the
    # fp16 rounding noise get exact logits recomputed on the host, making
    # the top-2 selection identical to full-precision routing.
    ls = np.sort(logits, axis=1)[:, ::-1]
    margin = np.minimum(ls[:, 0] - ls[:, 1], ls[:, 1] - ls[:, 2])
    fix = margin < 8e-3
    if fix.any():
        logits[fix] = (xf[fix].astype(np.float64)
                       @ router_w.T.astype(np.float64)).astype(np.float32)
    global _LAST_LOGITS
    _LAST_LOGITS = logits

    # --- host dispatch ---
    t1, t2, w1, w2 = _routing_from_logits(logits)
    idx_e = []
    wv_e = []
    for e in range(N_EXPERTS):
        sel = np.where((t1 == e) | (t2 == e))[0]
        idx_e.append(sel)
        wv_e.append(np.where(t1[sel] == e, w1[sel], w2[sel]).astype(np.float32))

    # --- launch 2: expert FFN on device ---
    np1 = np.float32 if EXPERT_MM1 == "f32r" else np.float16
    np2 = np.float32 if EXPERT_MM2 == "f32r" else np.float16
    out = np.zeros((B * T, D), np.float32)
    counts = np.array([len(s) for s in idx_e])
    order = np.argsort(-counts, kind="stable")
    use_split = (EXPERT_SPLIT and counts[order[0]] <= CAP_A
                 and counts[order[4:]].max() <= CAP_B)

    if use_split:
        # F-split pairing: the 4 hottest experts occupy slot A (CAP_A),
        # the 4 coldest slot B (CAP_B); each expert's two F-halves live on
        # two cores, host sums the partial y's.
        nc_e = _get_module("expert2")
        bigs, smalls = list(order[:4]), list(order[4:])
        xg_e, wv_g = {}, {}
        for e in range(N_EXPERTS):
            cap = CAP_A if e in bigs else CAP_B
            sl = idx_e[e]
            xg = np.zeros((D, cap), np1)
            xg[:, :len(sl)] = xT16[:, sl] if np1 == np.float16 else xT[:, sl]
            wvg = np.zeros((cap, 1), np.float32)
            wvg[:len(sl), 0] = wv_e[e]
            xg_e[e], wv_g[e] = xg, wvg
        in_maps = []
        for i in range(N_CORES):
            ea, eb, h = bigs[i // 2], smalls[i // 2], i % 2
            in_maps.append({
                "xTa": xg_e[ea],
                "w1Ta": np.ascontiguousarray(
                    fc1_w[ea].T[:, h * FH:(h + 1) * FH].astype(np1)),
                "w2Ta": np.ascontiguousarray(
                    fc2_w[ea].T[h * FH:(h + 1) * FH, :].astype(np2)),
                "wva": wv_g[ea],
                "xTb": xg_e[eb],
                "w1Tb": np.ascontiguousarray(
                    fc1_w[eb].T[:, h * FH:(h + 1) * FH].astype(np1)),
                "w2Tb": np.ascontiguousarray(
                    fc2_w[eb].T[h * FH:(h + 1) * FH, :].astype(np2)),
                "wvb": wv_g[eb],
            })
        res = run_bass_kernel_spmd(nc_e, in_maps, core_ids=list(range(N_CORES)))
        # combine in ascending expert order (reference accumulation order)
        for e in range(N_EXPERTS):
            sl = idx_e[e]
            if e in bigs:
                j = bigs.index(e)
                y0 = res.results[2 * j]["ya"]
                y1 = res.results[2 * j + 1]["ya"]
            else:
                j = smalls.index(e)
                y0 = res.results[2 * j]["yb"]
                y1 = res.results[2 * j + 1]["yb"]
            out[sl] += y0[:len(sl)] + y1[:len(sl)]
        return out.reshape(B, T, D)

    nc_e = _get_module("expert")
    w1T_np = [np.ascontiguousarray(fc1_w[e].T).astype(np1) for e in range(N_EXPERTS)]
    w2T_np = [np.ascontiguousarray(fc2_w[e].T).astype(np2) for e in range(N_EXPERTS)]
    n_passes = max(1, -(-max(len(s) for s in idx_e) // CAP))
    for p in range(n_passes):  # overflow fallback: extra passes never trigger
        in_maps = []           # for the fixed problem size (max count 2175)
        for e in range(N_EXPERTS):
            sl = idx_e[e][p * CAP:(p + 1) * CAP]
            xg = np.zeros((D, CAP), np1)
            xg[:, :len(sl)] = xT[:, sl].astype(np1)
            wvg = np.zeros((CAP, 1), np.float32)
            wvg[:len(sl), 0] = wv_e[e][p * CAP:(p + 1) * CAP]
            in_maps.append({"xT": xg, "w1T": w1T_np[e], "w2T": w2T_np[e],
                            "wv": wvg})
        res = run_bass_kernel_spmd(nc_e, in_maps, core_ids=list(range(N_CORES)))
        # host combine (ascending expert order == reference accumulation order)
        for e in range(N_EXPERTS):
            sl = idx_e[e][p * CAP:(p + 1) * CAP]
            out[sl] += res.results[e]["y"][:len(sl)]
    return out.reshape(B, T, D)

